# revision 13
# baseline (speedup 1.0000x reference)
"""CQVAE loss kernel for Trainium2, data-parallel over batch on 8 NeuronCores.

loss = kld(qy) + mse(gather(rzs), zs[:, :Sg]) + bias(best, best_gt)
       + bias(gather(pts), gts)
where bias(p, g) = mse(p, g) + 10 * mse(p[..., MARK, :], g[..., MARK, :]).

Each core handles 16 of the 128 batches.  The mapping-gathers run as
dma_gather ops (hundreds of rows per op, ~9ns/row of Q7 emission)
interleaved so gather bytes, zs bytes and compute pipeline smoothly.
pts/gts rows are zero-padded to 256 floats on the host so gathered rows
are 1KB-aligned and pad columns contribute nothing to the sums.  zs/gts
are laid out so every partition reads one contiguous 64/16KB run.  Each
core ships a [128, 32] per-partition stats tile; the host folds
partitions and cores.
"""

import sys

import numpy as np

try:
    import concourse  # noqa: F401
except ImportError:  # pragma: no cover
    sys.path.insert(0, "/opt/trn_rl_repo")

import concourse.bass as bass  # noqa: F401
import concourse.mybir as mybir
import concourse.tile as tile
from concourse import bacc, library_config
from concourse.bass_utils import run_bass_kernel_spmd

F32 = mybir.dt.float32
I16 = mybir.dt.int16
AX = mybir.AxisListType
OP = mybir.AluOpType
ACTF = mybir.ActivationFunctionType

NCORES = 8
B, S, SG, D, P, V = 128, 256, 128, 1024, 118, 64
BL = B // NCORES  # batches per core
P2 = 2 * P  # 236 true floats per point-row
PC = 256  # padded point-row width
MARK = (0, 29, 88, 117)
EPS = 1e-20
ALPHA = 10.0

NSTAT = 32
# stats columns
C_KLD = 28
C_BEST, C_BESTM = 29, 30
C_AE = 0  # 8 cols: ae pieces
C_BIAS = 8  # 4 cols: bias sq totals per pts quarter
C_MARK = 12  # 16 cols: 4 marks x 4 quarters

NAE = 8  # rzs gather ops / zs pieces (2 batches each)
NPT = 4  # pts gather ops / gts quarters (4 batches each)
KA = BL // NAE  # 2 batch-slots per ae piece
KP = BL // NPT  # 4 batch-slots per pts quarter

_module = None
last_results = None  # BassKernelResults of the most recent run (for profiling)


def _build_module():
    nc = bacc.Bacc()

    zs = nc.dram_tensor("zs", [BL * SG, D], F32, kind="ExternalInput")
    rzs = nc.dram_tensor("rzs", [BL * S, D], F32, kind="ExternalInput")
    pts = nc.dram_tensor("pts", [BL * S, PC], F32, kind="ExternalInput")
    gts = nc.dram_tensor("gts", [BL * SG, PC], F32, kind="ExternalInput")
    qy = nc.dram_tensor("qy", [BL * S, V], F32, kind="ExternalInput")
    best = nc.dram_tensor("best", [BL, P2], F32, kind="ExternalInput")
    best_gt = nc.dram_tensor("best_gt", [BL, P2], F32, kind="ExternalInput")
    # dma_gather index lists, int16, wrapped [p, s] = lin[s*16 + p%16]:
    # cols 0..127   eight rzs gathers (256 idxs each, 16 cols per op)
    # cols 128..255 four pts gathers (512 idxs each, 32 cols per op)
    idx2 = nc.dram_tensor("idx2", [128, 256], I16, kind="ExternalInput")
    out = nc.dram_tensor("out", [128, NSTAT], F32, kind="ExternalOutput")

    QCOLS = BL * S * V // 128  # 2048
    QN = BL * S // 128  # 32 qy rows per partition

    with tile.TileContext(nc) as tc:
        with tc.tile_pool(name="cst", bufs=1) as cst:
            nc.gpsimd.load_library(library_config.mlp)
            idx_t = cst.tile([128, 256], I16)
            nc.sync.dma_start(idx_t[:], idx2[:])

            stats = cst.tile([128, NSTAT], F32)
            nc.vector.memset(stats[:], 0.0)

            # ---- direct loads --------------------------------------------
            # scalar HWDGE queue: qy, best, gts quarters (3.2 MB)
            qy_t = cst.tile([128, QCOLS], F32)
            nc.scalar.dma_start(
                qy_t[:], qy[:].rearrange("(p n) v -> p (n v)", n=QN)
            )
            bt = cst.tile([BL, P2], F32)
            nc.scalar.dma_start(bt[:], best[:])
            bgt = cst.tile([BL, P2], F32)
            nc.scalar.dma_start(bgt[:], best_gt[:])
            # partition p holds gts rows 16p..16p+15 (contiguous 16KB)
            gts_r = gts[:].rearrange("(p k) c -> p (k c)", k=BL)
            gt_h = []
            for h in range(NPT):
                g = cst.tile([128, KP * PC], F32, tag=f"gt{h}", name=f"gt{h}")
                nc.scalar.dma_start(g[:], gts_r[:, h * KP * PC : (h + 1) * KP * PC])
                gt_h.append(g)

            # sync HWDGE queue: zs pieces (8.4 MB)
            # partition p holds zs rows 16p..16p+15 (contiguous 64KB)
            zs_r = zs[:].rearrange("(p k) d -> p (k d)", k=BL)
            zs_t = []
            for j in range(NAE):
                z = cst.tile([128, KA * D], F32, tag=f"zs{j}", name=f"zs{j}")
                nc.sync.dma_start(z[:], zs_r[:, j * KA * D : (j + 1) * KA * D])
                zs_t.append(z)

            # ---- gathers (SWDGE queue 0, dma_gather) ----------------------
            rg_t = []
            for j in range(NAE):
                rg = cst.tile([128, KA * D], F32, tag=f"rg{j}", name=f"rg{j}")
                rg_t.append(rg)
            pg_t = []
            for h in range(NPT):
                pg = cst.tile([128, KP * PC], F32, tag=f"pg{h}", name=f"pg{h}")
                pg_t.append(pg)

            def gather_rzs(j):
                nidx = KA * 128  # 256
                nc.gpsimd.dma_gather(
                    rg_t[j][:].rearrange("p (k d) -> p k d", d=D),
                    rzs[:],
                    idx_t[:, j * 16 : (j + 1) * 16],
                    nidx,
                    nidx,
                    D,
                )

            def gather_pts(h):
                nidx = KP * 128  # 512
                nc.gpsimd.dma_gather(
                    pg_t[h][:].rearrange("p (k c) -> p k c", c=PC),
                    pts[:],
                    idx_t[:, 128 + h * 32 : 128 + (h + 1) * 32],
                    nidx,
                    nidx,
                    PC,
                )

            # all rzs gathers first: their ~385 GB/s emission builds ring
            # backlog that keeps queue 0 draining through the slower
            # (~110 GB/s) pts-descriptor emission at the end
            for j in range(NAE):
                gather_rzs(j)
            for h in range(NPT):
                gather_pts(h)

            # ---- compute --------------------------------------------------
            # BEST (tiny, lands early on the scalar queue)
            nc.vector.tensor_sub(bt[:], bt[:], bgt[:])
            nc.vector.tensor_mul(bt[:], bt[:], bt[:])
            nc.vector.reduce_sum(out=stats[:BL, C_BEST : C_BEST + 1], in_=bt[:], axis=AX.X)
            bm4 = cst.tile([BL, 4], F32)
            for j, m in enumerate(MARK):
                nc.vector.reduce_sum(
                    out=bm4[:, j : j + 1], in_=bt[:, 2 * m : 2 * m + 2], axis=AX.X
                )
            nc.vector.reduce_sum(out=stats[:BL, C_BESTM : C_BESTM + 1], in_=bm4[:], axis=AX.X)

            # KLD: sum q * (log(q + eps) - log(1/V)) via log(V*q + V*eps)
            lg = cst.tile([128, QCOLS], F32)
            ebias = cst.tile([128, 1], F32)
            nc.vector.memset(ebias[:], float(V) * EPS)
            nc.scalar.activation(lg[:], qy_t[:], ACTF.Ln, bias=ebias[:], scale=float(V))
            nc.vector.scalar_tensor_tensor(
                out=lg[:],
                in0=lg[:],
                scalar=0.0,
                in1=qy_t[:],
                op0=OP.subtract,
                op1=OP.mult,
                accum_out=stats[:, C_KLD : C_KLD + 1],
            )

            def ae_piece(j):
                nc.vector.tensor_sub(rg_t[j][:], rg_t[j][:], zs_t[j][:])
                nc.scalar.activation(
                    rg_t[j][:], rg_t[j][:], ACTF.Square,
                    accum_out=stats[:, C_AE + j : C_AE + j + 1],
                )

            def bias_quarter(h):
                nc.vector.tensor_sub(pg_t[h][:], pg_t[h][:], gt_h[h][:])
                nc.scalar.activation(
                    pg_t[h][:], pg_t[h][:], ACTF.Square,
                    accum_out=stats[:, C_BIAS + h : C_BIAS + h + 1],
                )
                sq3 = pg_t[h][:].rearrange("p (k c) -> p k c", c=PC)
                cm = C_MARK + 4 * h
                for j, m in enumerate(MARK):
                    nc.vector.reduce_sum(
                        out=stats[:, cm + j : cm + j + 1],
                        in_=sq3[:, :, 2 * m : 2 * m + 2],
                        axis=AX.XY,
                    )

            # compute in data-arrival order: rzs pieces stream in first,
            # pts quarters land last
            for j in range(NAE):
                ae_piece(j)
            for h in range(NPT):
                bias_quarter(h)

            nc.sync.dma_start(out[:], stats[:])

    nc.compile()
    return nc


def kernel(
    zs, rzs, pts, best, qy, gts, best_gt, mapping, vector_dims, **trace_kwargs
):
    global _module, last_results
    vd = int(np.asarray(vector_dims))
    assert vd == V, f"kernel compiled for vector_dims={V}, got {vd}"

    if _module is None:
        _module = _build_module()

    zs = np.asarray(zs, dtype=np.float32)
    rzs = np.asarray(rzs, dtype=np.float32)
    pts = np.asarray(pts, dtype=np.float32)
    gts = np.asarray(gts, dtype=np.float32)
    qy = np.asarray(qy, dtype=np.float32)
    mapping = np.asarray(mapping).astype(np.int32)
    best2 = np.ascontiguousarray(np.asarray(best, dtype=np.float32).reshape(B, P2))
    bgt2 = np.ascontiguousarray(np.asarray(best_gt, dtype=np.float32).reshape(B, P2))

    # zero-pad point rows to PC floats
    pts_p = np.zeros((B, S, PC), dtype=np.float32)
    pts_p[:, :, :P2] = pts.reshape(B, S, P2)
    gts_p = np.zeros((B, SG, PC), dtype=np.float32)
    gts_p[:, :, :P2] = gts.reshape(B, SG, P2)

    def wrap16(lin):
        # dma_gather index layout: idxs[p, s] = lin[s*16 + p%16]
        return np.tile(lin.reshape(-1, 16).T, (8, 1))

    # partition p <-> (b = p//8, q = p%8); slot k within a piece
    kk_a, pp_a = np.divmod(np.arange(KA * 128), 128)  # rzs ops
    kk_p, pp_p = np.divmod(np.arange(KP * 128), 128)  # pts ops
    in_maps = []
    for c in range(NCORES):
        sl = slice(c * BL, (c + 1) * BL)
        mp = mapping[sl]  # [BL, SG]
        blocks = []
        for j in range(NAE):  # dst[p, k] = rzs[b, mapping[b, 16q + KA*j + k]]
            b = pp_a // 8
            pos = 16 * (pp_a % 8) + KA * j + kk_a
            blocks.append(wrap16(b * S + mp[b, pos]))
        for h in range(NPT):  # dst[p, k] matches gts rows 16p + KP*h + k
            b = pp_p // 8
            pos = 16 * (pp_p % 8) + KP * h + kk_p
            blocks.append(wrap16(b * S + mp[b, pos]))
        idx2 = np.concatenate(blocks, axis=1).astype(np.int16)
        # zs rows reordered so partition p holds rows 16p..16p+15:
        # row 16p+k = zs[b, 16q+k] -> natural order already (b-major, i-minor)
        in_maps.append(
            {
                "zs": np.ascontiguousarray(zs[sl, :SG].reshape(BL * SG, D)),
                "rzs": rzs[sl].reshape(BL * S, D),
                "pts": pts_p[sl].reshape(BL * S, PC),
                "gts": gts_p[sl].reshape(BL * SG, PC),
                "qy": qy[sl].reshape(BL * S, V),
                "best": np.ascontiguousarray(best2[sl]),
                "best_gt": np.ascontiguousarray(bgt2[sl]),
                "idx2": np.ascontiguousarray(idx2),
            }
        )

    last_results = run_bass_kernel_spmd(
        _module, in_maps, list(range(NCORES)), **trace_kwargs
    )
    parts = np.stack(
        [
            np.asarray(r["out"], dtype=np.float64).reshape(128, NSTAT).sum(axis=0)
            for r in last_results.results
        ]
    )
    tot = parts.sum(axis=0)

    ae_loss = tot[C_AE : C_AE + NAE].sum() / (B * SG * D)
    bias_sq = tot[C_BIAS : C_BIAS + NPT].sum()
    mark_sq = tot[C_MARK : C_MARK + 4 * NPT].sum()
    bias_loss = bias_sq / (B * SG * P2) + ALPHA * mark_sq / (B * SG * 2 * len(MARK))
    kld_loss = tot[C_KLD] / (B * S)
    best_mse = tot[C_BEST] / (B * P2) + ALPHA * tot[C_BESTM] / (B * 2 * len(MARK))

    return np.array(kld_loss + ae_loss + best_mse + bias_loss, dtype=np.float32)


# revision 16
# speedup vs baseline: 1.0592x; 1.0592x over previous
"""CQVAE loss kernel for Trainium2, data-parallel over batch on 8 NeuronCores.

loss = kld(qy) + mse(gather(rzs), zs[:, :Sg]) + bias(best, best_gt)
       + bias(gather(pts), gts)
where bias(p, g) = mse(p, g) + 10 * mse(p[..., MARK, :], g[..., MARK, :]).

Each core handles 16 of the 128 batches.  The mapping-gathers run as
dma_gather ops (hundreds of rows per op, ~9ns/row of Q7 emission)
interleaved so gather bytes, zs bytes and compute pipeline smoothly.
pts/gts rows are zero-padded to 256 floats on the host so gathered rows
are 1KB-aligned and pad columns contribute nothing to the sums.  zs/gts
are laid out so every partition reads one contiguous 64/16KB run.  Each
core ships a [128, 32] per-partition stats tile; the host folds
partitions and cores.
"""

import sys

import numpy as np

try:
    import concourse  # noqa: F401
except ImportError:  # pragma: no cover
    sys.path.insert(0, "/opt/trn_rl_repo")

import concourse.bass as bass  # noqa: F401
import concourse.mybir as mybir
import concourse.tile as tile
from concourse import bacc, library_config
from concourse.bass_utils import run_bass_kernel_spmd

F32 = mybir.dt.float32
I16 = mybir.dt.int16
AX = mybir.AxisListType
OP = mybir.AluOpType
ACTF = mybir.ActivationFunctionType

NCORES = 8
B, S, SG, D, P, V = 128, 256, 128, 1024, 118, 64
BL = B // NCORES  # batches per core
P2 = 2 * P  # 236 true floats per point-row
PC = 256  # padded point-row width
MARK = (0, 29, 88, 117)
EPS = 1e-20
ALPHA = 10.0

NSTAT = 36
# stats columns
C_KLD = 33
C_BEST, C_BESTM = 10, 11
C_AE = 0  # 10 cols: ae pieces
C_BIAS = 12  # 4 cols: bias sq totals per pts quarter
C_MARK = 16  # 16 cols: 4 marks x 4 quarters

# rzs pieces by (start_slot, n_slots): coarse early, 1-slot at the end
AE_PIECES = [(0, 2), (2, 2), (4, 2), (6, 2), (8, 2), (10, 2),
             (12, 1), (13, 1), (14, 1), (15, 1)]
NAE = len(AE_PIECES)
NPT = 4  # pts gather ops / gts quarters (4 batches each)
KP = BL // NPT  # 4 batch-slots per pts quarter

_module = None
last_results = None  # BassKernelResults of the most recent run (for profiling)


def _build_module():
    nc = bacc.Bacc()

    zs = nc.dram_tensor("zs", [BL * SG, D], F32, kind="ExternalInput")
    rzs = nc.dram_tensor("rzs", [BL * S, D], F32, kind="ExternalInput")
    pts = nc.dram_tensor("pts", [BL * S, PC], F32, kind="ExternalInput")
    gts = nc.dram_tensor("gts", [BL * SG, PC], F32, kind="ExternalInput")
    qy = nc.dram_tensor("qy", [BL * S, V], F32, kind="ExternalInput")
    best = nc.dram_tensor("best", [BL, P2], F32, kind="ExternalInput")
    best_gt = nc.dram_tensor("best_gt", [BL, P2], F32, kind="ExternalInput")
    # dma_gather index lists, int16, wrapped [p, s] = lin[s*16 + p%16]:
    # cols 0..127   eight rzs gathers (256 idxs each, 16 cols per op)
    # cols 128..255 four pts gathers (512 idxs each, 32 cols per op)
    idx2 = nc.dram_tensor("idx2", [128, 256], I16, kind="ExternalInput")
    out = nc.dram_tensor("out", [128, NSTAT], F32, kind="ExternalOutput")

    QCOLS = BL * S * V // 128  # 2048
    QN = BL * S // 128  # 32 qy rows per partition

    with tile.TileContext(nc) as tc:
        with tc.tile_pool(name="cst", bufs=1) as cst:
            nc.gpsimd.load_library(library_config.mlp)
            idx_t = cst.tile([128, 256], I16)
            nc.sync.dma_start(idx_t[:], idx2[:])

            stats = cst.tile([128, NSTAT], F32)
            nc.vector.memset(stats[:], 0.0)

            # ---- gathers (SWDGE queue 0, dma_gather) ----------------------
            rg_t = []
            for j, (s0, ns) in enumerate(AE_PIECES):
                rg = cst.tile([128, ns * D], F32, tag=f"rg{j}", name=f"rg{j}")
                rg_t.append(rg)
            pg_t = []
            for h in range(NPT):
                pg = cst.tile([128, KP * PC], F32, tag=f"pg{h}", name=f"pg{h}")
                pg_t.append(pg)

            def gather_rzs(j):
                s0, ns = AE_PIECES[j]
                nidx = ns * 128
                nc.gpsimd.dma_gather(
                    rg_t[j][:].rearrange("p (k d) -> p k d", d=D),
                    rzs[:],
                    idx_t[:, s0 * 8 : (s0 + ns) * 8],
                    nidx,
                    nidx,
                    D,
                )

            def gather_pts(h):
                nidx = KP * 128  # 512
                nc.gpsimd.dma_gather(
                    pg_t[h][:].rearrange("p (k c) -> p k c", c=PC),
                    pts[:],
                    idx_t[:, 128 + h * 32 : 128 + (h + 1) * 32],
                    nidx,
                    nidx,
                    PC,
                )

            GORDER = [("r", 0), ("r", 1), ("p", 0), ("r", 2), ("r", 3),
                      ("p", 1), ("r", 4), ("r", 5), ("p", 2), ("r", 6),
                      ("r", 7), ("r", 8), ("r", 9), ("p", 3)]
            for kind, i in GORDER:
                if kind == "r":
                    gather_rzs(i)
                else:
                    gather_pts(i)

            # ---- direct loads --------------------------------------------
            # scalar HWDGE queue: qy, best, gts quarters (3.2 MB)
            qy_t = cst.tile([128, QCOLS], F32)
            nc.scalar.dma_start(
                qy_t[:], qy[:].rearrange("(p n) v -> p (n v)", n=QN)
            )
            bt = cst.tile([BL, P2], F32)
            nc.scalar.dma_start(bt[:], best[:])
            bgt = cst.tile([BL, P2], F32)
            nc.scalar.dma_start(bgt[:], best_gt[:])
            # partition p holds gts rows 16p..16p+15 (contiguous 16KB)
            gts_r = gts[:].rearrange("(p k) c -> p (k c)", k=BL)
            gt_h = []
            for h in range(NPT):
                g = cst.tile([128, KP * PC], F32, tag=f"gt{h}", name=f"gt{h}")
                nc.scalar.dma_start(g[:], gts_r[:, h * KP * PC : (h + 1) * KP * PC])
                gt_h.append(g)

            # sync HWDGE queue: zs pieces (8.4 MB)
            # partition p holds zs rows 16p..16p+15 (contiguous 64KB)
            zs_r = zs[:].rearrange("(p k) d -> p (k d)", k=BL)
            zs_t = []
            for j, (s0, ns) in enumerate(AE_PIECES):
                z = cst.tile([128, ns * D], F32, tag=f"zs{j}", name=f"zs{j}")
                nc.sync.dma_start(z[:], zs_r[:, s0 * D : (s0 + ns) * D])
                zs_t.append(z)

            # ---- compute --------------------------------------------------
            # BEST (tiny, lands early on the scalar queue)
            nc.vector.tensor_sub(bt[:], bt[:], bgt[:])
            nc.vector.tensor_mul(bt[:], bt[:], bt[:])
            nc.vector.reduce_sum(out=stats[:BL, C_BEST : C_BEST + 1], in_=bt[:], axis=AX.X)
            bm4 = cst.tile([BL, 4], F32)
            for j, m in enumerate(MARK):
                nc.vector.reduce_sum(
                    out=bm4[:, j : j + 1], in_=bt[:, 2 * m : 2 * m + 2], axis=AX.X
                )
            nc.vector.reduce_sum(out=stats[:BL, C_BESTM : C_BESTM + 1], in_=bm4[:], axis=AX.X)

            # KLD: sum q * (log(q + eps) - log(1/V)) via log(V*q + V*eps)
            lg = cst.tile([128, QCOLS], F32)
            ebias = cst.tile([128, 1], F32)
            nc.vector.memset(ebias[:], float(V) * EPS)
            nc.scalar.activation(lg[:], qy_t[:], ACTF.Ln, bias=ebias[:], scale=float(V))
            nc.vector.scalar_tensor_tensor(
                out=lg[:],
                in0=lg[:],
                scalar=0.0,
                in1=qy_t[:],
                op0=OP.subtract,
                op1=OP.mult,
                accum_out=stats[:, C_KLD : C_KLD + 1],
            )

            def ae_piece(j):
                nc.vector.tensor_sub(rg_t[j][:], rg_t[j][:], zs_t[j][:])
                nc.scalar.activation(
                    rg_t[j][:], rg_t[j][:], ACTF.Square,
                    accum_out=stats[:, C_AE + j : C_AE + j + 1],
                )

            def bias_quarter(h):
                nc.vector.tensor_sub(pg_t[h][:], pg_t[h][:], gt_h[h][:])
                nc.scalar.activation(
                    pg_t[h][:], pg_t[h][:], ACTF.Square,
                    accum_out=stats[:, C_BIAS + h : C_BIAS + h + 1],
                )
                sq3 = pg_t[h][:].rearrange("p (k c) -> p k c", c=PC)
                cm = C_MARK + 4 * h
                for j, m in enumerate(MARK):
                    nc.vector.reduce_sum(
                        out=stats[:, cm + j : cm + j + 1],
                        in_=sq3[:, :, 2 * m : 2 * m + 2],
                        axis=AX.XY,
                    )

            # compute in data-arrival order
            ae_piece(0)
            ae_piece(1)
            bias_quarter(0)
            ae_piece(2)
            ae_piece(3)
            bias_quarter(1)
            ae_piece(4)
            ae_piece(5)
            bias_quarter(2)
            ae_piece(6)
            ae_piece(7)
            ae_piece(8)
            ae_piece(9)
            bias_quarter(3)

            nc.sync.dma_start(out[:], stats[:])

    nc.compile()
    return nc


def kernel(
    zs, rzs, pts, best, qy, gts, best_gt, mapping, vector_dims, **trace_kwargs
):
    global _module, last_results
    vd = int(np.asarray(vector_dims))
    assert vd == V, f"kernel compiled for vector_dims={V}, got {vd}"

    if _module is None:
        _module = _build_module()

    zs = np.asarray(zs, dtype=np.float32)
    rzs = np.asarray(rzs, dtype=np.float32)
    pts = np.asarray(pts, dtype=np.float32)
    gts = np.asarray(gts, dtype=np.float32)
    qy = np.asarray(qy, dtype=np.float32)
    mapping = np.asarray(mapping).astype(np.int32)
    best2 = np.ascontiguousarray(np.asarray(best, dtype=np.float32).reshape(B, P2))
    bgt2 = np.ascontiguousarray(np.asarray(best_gt, dtype=np.float32).reshape(B, P2))

    # zero-pad point rows to PC floats
    pts_p = np.zeros((B, S, PC), dtype=np.float32)
    pts_p[:, :, :P2] = pts.reshape(B, S, P2)
    gts_p = np.zeros((B, SG, PC), dtype=np.float32)
    gts_p[:, :, :P2] = gts.reshape(B, SG, P2)

    def wrap16(lin):
        # dma_gather index layout: idxs[p, s] = lin[s*16 + p%16]
        return np.tile(lin.reshape(-1, 16).T, (8, 1))

    # partition p <-> (b = p//8, q = p%8); slot k within a piece
    kk_p, pp_p = np.divmod(np.arange(KP * 128), 128)  # pts ops
    in_maps = []
    for c in range(NCORES):
        sl = slice(c * BL, (c + 1) * BL)
        mp = mapping[sl]  # [BL, SG]
        blocks = []
        for s0, ns in AE_PIECES:  # dst[p, k] = rzs[b, mapping[b, 16q + s0 + k]]
            kk_a, pp_a = np.divmod(np.arange(ns * 128), 128)
            b = pp_a // 8
            pos = 16 * (pp_a % 8) + s0 + kk_a
            blocks.append(wrap16(b * S + mp[b, pos]))
        for h in range(NPT):  # dst[p, k] matches gts rows 16p + KP*h + k
            b = pp_p // 8
            pos = 16 * (pp_p % 8) + KP * h + kk_p
            blocks.append(wrap16(b * S + mp[b, pos]))
        idx2 = np.concatenate(blocks, axis=1).astype(np.int16)
        # zs rows reordered so partition p holds rows 16p..16p+15:
        # row 16p+k = zs[b, 16q+k] -> natural order already (b-major, i-minor)
        in_maps.append(
            {
                "zs": np.ascontiguousarray(zs[sl, :SG].reshape(BL * SG, D)),
                "rzs": rzs[sl].reshape(BL * S, D),
                "pts": pts_p[sl].reshape(BL * S, PC),
                "gts": gts_p[sl].reshape(BL * SG, PC),
                "qy": qy[sl].reshape(BL * S, V),
                "best": np.ascontiguousarray(best2[sl]),
                "best_gt": np.ascontiguousarray(bgt2[sl]),
                "idx2": np.ascontiguousarray(idx2),
            }
        )

    last_results = run_bass_kernel_spmd(
        _module, in_maps, list(range(NCORES)), **trace_kwargs
    )
    parts = np.stack(
        [
            np.asarray(r["out"], dtype=np.float64).reshape(128, NSTAT).sum(axis=0)
            for r in last_results.results
        ]
    )
    tot = parts.sum(axis=0)

    ae_loss = tot[C_AE : C_AE + NAE].sum() / (B * SG * D)
    bias_sq = tot[C_BIAS : C_BIAS + NPT].sum()
    mark_sq = tot[C_MARK : C_MARK + 4 * NPT].sum()
    bias_loss = bias_sq / (B * SG * P2) + ALPHA * mark_sq / (B * SG * 2 * len(MARK))
    kld_loss = tot[C_KLD] / (B * S)
    best_mse = tot[C_BEST] / (B * P2) + ALPHA * tot[C_BESTM] / (B * 2 * len(MARK))

    return np.array(kld_loss + ae_loss + best_mse + bias_loss, dtype=np.float32)


# revision 17
# speedup vs baseline: 1.0646x; 1.0051x over previous
"""CQVAE loss kernel for Trainium2, data-parallel over batch on 8 NeuronCores.

loss = kld(qy) + mse(gather(rzs), zs[:, :Sg]) + bias(best, best_gt)
       + bias(gather(pts), gts)
where bias(p, g) = mse(p, g) + 10 * mse(p[..., MARK, :], g[..., MARK, :]).

Each core handles 16 of the 128 batches.  The mapping-gathers run as
dma_gather ops (hundreds of rows per op, ~9ns/row of Q7 emission)
interleaved so gather bytes, zs bytes and compute pipeline smoothly.
pts/gts rows are zero-padded to 256 floats on the host so gathered rows
are 1KB-aligned and pad columns contribute nothing to the sums.  zs/gts
are laid out so every partition reads one contiguous 64/16KB run.  Each
core ships a [128, 32] per-partition stats tile; the host folds
partitions and cores.
"""

import sys

import numpy as np

try:
    import concourse  # noqa: F401
except ImportError:  # pragma: no cover
    sys.path.insert(0, "/opt/trn_rl_repo")

import concourse.bass as bass  # noqa: F401
import concourse.mybir as mybir
import concourse.tile as tile
from concourse import bacc, library_config
from concourse.bass_utils import run_bass_kernel_spmd

F32 = mybir.dt.float32
I16 = mybir.dt.int16
AX = mybir.AxisListType
OP = mybir.AluOpType
ACTF = mybir.ActivationFunctionType

NCORES = 8
B, S, SG, D, P, V = 128, 256, 128, 1024, 118, 64
BL = B // NCORES  # batches per core
P2 = 2 * P  # 236 true floats per point-row
PC = 256  # padded point-row width
MARK = (0, 29, 88, 117)
EPS = 1e-20
ALPHA = 10.0

NSTAT = 36
# stats columns
C_KLD = 33
C_BEST, C_BESTM = 10, 11
C_AE = 0  # 10 cols: ae pieces
C_BIAS = 12  # 4 cols: bias sq totals per pts quarter
C_MARK = 16  # 16 cols: 4 marks x 4 quarters

# rzs pieces by (start_slot, n_slots): coarse early, 1-slot at the end
AE_PIECES = [(0, 2), (2, 2), (4, 2), (6, 2), (8, 2), (10, 2),
             (12, 1), (13, 1), (14, 1), (15, 1)]
NAE = len(AE_PIECES)
NPT = 4  # pts gather ops / gts quarters (4 batches each)
KP = BL // NPT  # 4 batch-slots per pts quarter

_module = None
last_results = None  # BassKernelResults of the most recent run (for profiling)


def _build_module():
    nc = bacc.Bacc()

    zs = nc.dram_tensor("zs", [BL * SG, D], F32, kind="ExternalInput")
    rzs = nc.dram_tensor("rzs", [BL * S, D], F32, kind="ExternalInput")
    pts = nc.dram_tensor("pts", [BL * S, PC], F32, kind="ExternalInput")
    gts = nc.dram_tensor("gts", [BL * SG, PC], F32, kind="ExternalInput")
    qy = nc.dram_tensor("qy", [BL * S, V], F32, kind="ExternalInput")
    best = nc.dram_tensor("best", [BL, P2], F32, kind="ExternalInput")
    best_gt = nc.dram_tensor("best_gt", [BL, P2], F32, kind="ExternalInput")
    # dma_gather index lists, int16, wrapped [p, s] = lin[s*16 + p%16]:
    # cols 0..127   eight rzs gathers (256 idxs each, 16 cols per op)
    # cols 128..255 four pts gathers (512 idxs each, 32 cols per op)
    idx2 = nc.dram_tensor("idx2", [128, 256], I16, kind="ExternalInput")
    out = nc.dram_tensor("out", [128, NSTAT], F32, kind="ExternalOutput")

    QCOLS = BL * S * V // 128  # 2048
    QN = BL * S // 128  # 32 qy rows per partition

    with tile.TileContext(nc) as tc:
        with tc.tile_pool(name="cst", bufs=1) as cst:
            nc.gpsimd.load_library(library_config.mlp)
            idx_t = cst.tile([128, 256], I16)
            nc.sync.dma_start(idx_t[:], idx2[:])

            stats = cst.tile([128, NSTAT], F32)
            nc.vector.memset(stats[:], 0.0)

            # ---- gathers (SWDGE queue 0, dma_gather) ----------------------
            rg_t = []
            for j, (s0, ns) in enumerate(AE_PIECES):
                rg = cst.tile([128, ns * D], F32, tag=f"rg{j}", name=f"rg{j}")
                rg_t.append(rg)
            pg_t = []
            for h in range(NPT):
                pg = cst.tile([128, KP * PC], F32, tag=f"pg{h}", name=f"pg{h}")
                pg_t.append(pg)

            def gather_rzs(j):
                s0, ns = AE_PIECES[j]
                nidx = ns * 128
                nc.gpsimd.dma_gather(
                    rg_t[j][:].rearrange("p (k d) -> p k d", d=D),
                    rzs[:],
                    idx_t[:, s0 * 8 : (s0 + ns) * 8],
                    nidx,
                    nidx,
                    D,
                )

            def gather_pts(h):
                nidx = KP * 128  # 512
                nc.gpsimd.dma_gather(
                    pg_t[h][:].rearrange("p (k c) -> p k c", c=PC),
                    pts[:],
                    idx_t[:, 128 + h * 32 : 128 + (h + 1) * 32],
                    nidx,
                    nidx,
                    PC,
                )

            GORDER = [("r", 0), ("r", 1), ("p", 0), ("r", 2), ("p", 1),
                      ("r", 3), ("p", 2), ("r", 4), ("p", 3), ("r", 5),
                      ("r", 6), ("r", 7), ("r", 8), ("r", 9)]
            for kind, i in GORDER:
                if kind == "r":
                    gather_rzs(i)
                else:
                    gather_pts(i)

            # ---- direct loads --------------------------------------------
            # scalar HWDGE queue: qy, best, gts quarters (3.2 MB)
            qy_t = cst.tile([128, QCOLS], F32)
            nc.scalar.dma_start(
                qy_t[:], qy[:].rearrange("(p n) v -> p (n v)", n=QN)
            )
            bt = cst.tile([BL, P2], F32)
            nc.scalar.dma_start(bt[:], best[:])
            bgt = cst.tile([BL, P2], F32)
            nc.scalar.dma_start(bgt[:], best_gt[:])
            # partition p holds gts rows 16p..16p+15 (contiguous 16KB)
            gts_r = gts[:].rearrange("(p k) c -> p (k c)", k=BL)
            gt_h = []
            for h in range(NPT):
                g = cst.tile([128, KP * PC], F32, tag=f"gt{h}", name=f"gt{h}")
                nc.scalar.dma_start(g[:], gts_r[:, h * KP * PC : (h + 1) * KP * PC])
                gt_h.append(g)

            # sync HWDGE queue: zs pieces (8.4 MB)
            # partition p holds zs rows 16p..16p+15 (contiguous 64KB)
            zs_r = zs[:].rearrange("(p k) d -> p (k d)", k=BL)
            zs_t = []
            for j, (s0, ns) in enumerate(AE_PIECES):
                z = cst.tile([128, ns * D], F32, tag=f"zs{j}", name=f"zs{j}")
                nc.sync.dma_start(z[:], zs_r[:, s0 * D : (s0 + ns) * D])
                zs_t.append(z)

            # ---- compute --------------------------------------------------
            # BEST (tiny, lands early on the scalar queue)
            nc.vector.tensor_sub(bt[:], bt[:], bgt[:])
            nc.vector.tensor_mul(bt[:], bt[:], bt[:])
            nc.vector.reduce_sum(out=stats[:BL, C_BEST : C_BEST + 1], in_=bt[:], axis=AX.X)
            bm4 = cst.tile([BL, 4], F32)
            for j, m in enumerate(MARK):
                nc.vector.reduce_sum(
                    out=bm4[:, j : j + 1], in_=bt[:, 2 * m : 2 * m + 2], axis=AX.X
                )
            nc.vector.reduce_sum(out=stats[:BL, C_BESTM : C_BESTM + 1], in_=bm4[:], axis=AX.X)

            # KLD: sum q * (log(q + eps) - log(1/V)) via log(V*q + V*eps)
            lg = cst.tile([128, QCOLS], F32)
            ebias = cst.tile([128, 1], F32)
            nc.vector.memset(ebias[:], float(V) * EPS)
            nc.scalar.activation(lg[:], qy_t[:], ACTF.Ln, bias=ebias[:], scale=float(V))
            nc.vector.scalar_tensor_tensor(
                out=lg[:],
                in0=lg[:],
                scalar=0.0,
                in1=qy_t[:],
                op0=OP.subtract,
                op1=OP.mult,
                accum_out=stats[:, C_KLD : C_KLD + 1],
            )

            def ae_piece(j):
                nc.vector.tensor_sub(rg_t[j][:], rg_t[j][:], zs_t[j][:])
                nc.scalar.activation(
                    rg_t[j][:], rg_t[j][:], ACTF.Square,
                    accum_out=stats[:, C_AE + j : C_AE + j + 1],
                )

            def bias_quarter(h):
                nc.vector.tensor_sub(pg_t[h][:], pg_t[h][:], gt_h[h][:])
                nc.scalar.activation(
                    pg_t[h][:], pg_t[h][:], ACTF.Square,
                    accum_out=stats[:, C_BIAS + h : C_BIAS + h + 1],
                )
                sq3 = pg_t[h][:].rearrange("p (k c) -> p k c", c=PC)
                cm = C_MARK + 4 * h
                for j, m in enumerate(MARK):
                    nc.vector.reduce_sum(
                        out=stats[:, cm + j : cm + j + 1],
                        in_=sq3[:, :, 2 * m : 2 * m + 2],
                        axis=AX.XY,
                    )

            # compute in data-arrival order
            ae_piece(0)
            ae_piece(1)
            bias_quarter(0)
            ae_piece(2)
            bias_quarter(1)
            ae_piece(3)
            bias_quarter(2)
            ae_piece(4)
            bias_quarter(3)
            ae_piece(5)
            ae_piece(6)
            ae_piece(7)
            ae_piece(8)
            ae_piece(9)

            nc.sync.dma_start(out[:], stats[:])

    nc.compile()
    return nc


def kernel(
    zs, rzs, pts, best, qy, gts, best_gt, mapping, vector_dims, **trace_kwargs
):
    global _module, last_results
    vd = int(np.asarray(vector_dims))
    assert vd == V, f"kernel compiled for vector_dims={V}, got {vd}"

    if _module is None:
        _module = _build_module()

    zs = np.asarray(zs, dtype=np.float32)
    rzs = np.asarray(rzs, dtype=np.float32)
    pts = np.asarray(pts, dtype=np.float32)
    gts = np.asarray(gts, dtype=np.float32)
    qy = np.asarray(qy, dtype=np.float32)
    mapping = np.asarray(mapping).astype(np.int32)
    best2 = np.ascontiguousarray(np.asarray(best, dtype=np.float32).reshape(B, P2))
    bgt2 = np.ascontiguousarray(np.asarray(best_gt, dtype=np.float32).reshape(B, P2))

    # zero-pad point rows to PC floats
    pts_p = np.zeros((B, S, PC), dtype=np.float32)
    pts_p[:, :, :P2] = pts.reshape(B, S, P2)
    gts_p = np.zeros((B, SG, PC), dtype=np.float32)
    gts_p[:, :, :P2] = gts.reshape(B, SG, P2)

    def wrap16(lin):
        # dma_gather index layout: idxs[p, s] = lin[s*16 + p%16]
        return np.tile(lin.reshape(-1, 16).T, (8, 1))

    # partition p <-> (b = p//8, q = p%8); slot k within a piece
    kk_p, pp_p = np.divmod(np.arange(KP * 128), 128)  # pts ops
    in_maps = []
    for c in range(NCORES):
        sl = slice(c * BL, (c + 1) * BL)
        mp = mapping[sl]  # [BL, SG]
        blocks = []
        for s0, ns in AE_PIECES:  # dst[p, k] = rzs[b, mapping[b, 16q + s0 + k]]
            kk_a, pp_a = np.divmod(np.arange(ns * 128), 128)
            b = pp_a // 8
            pos = 16 * (pp_a % 8) + s0 + kk_a
            blocks.append(wrap16(b * S + mp[b, pos]))
        for h in range(NPT):  # dst[p, k] matches gts rows 16p + KP*h + k
            b = pp_p // 8
            pos = 16 * (pp_p % 8) + KP * h + kk_p
            blocks.append(wrap16(b * S + mp[b, pos]))
        idx2 = np.concatenate(blocks, axis=1).astype(np.int16)
        # zs rows reordered so partition p holds rows 16p..16p+15:
        # row 16p+k = zs[b, 16q+k] -> natural order already (b-major, i-minor)
        in_maps.append(
            {
                "zs": np.ascontiguousarray(zs[sl, :SG].reshape(BL * SG, D)),
                "rzs": rzs[sl].reshape(BL * S, D),
                "pts": pts_p[sl].reshape(BL * S, PC),
                "gts": gts_p[sl].reshape(BL * SG, PC),
                "qy": qy[sl].reshape(BL * S, V),
                "best": np.ascontiguousarray(best2[sl]),
                "best_gt": np.ascontiguousarray(bgt2[sl]),
                "idx2": np.ascontiguousarray(idx2),
            }
        )

    last_results = run_bass_kernel_spmd(
        _module, in_maps, list(range(NCORES)), **trace_kwargs
    )
    parts = np.stack(
        [
            np.asarray(r["out"], dtype=np.float64).reshape(128, NSTAT).sum(axis=0)
            for r in last_results.results
        ]
    )
    tot = parts.sum(axis=0)

    ae_loss = tot[C_AE : C_AE + NAE].sum() / (B * SG * D)
    bias_sq = tot[C_BIAS : C_BIAS + NPT].sum()
    mark_sq = tot[C_MARK : C_MARK + 4 * NPT].sum()
    bias_loss = bias_sq / (B * SG * P2) + ALPHA * mark_sq / (B * SG * 2 * len(MARK))
    kld_loss = tot[C_KLD] / (B * S)
    best_mse = tot[C_BEST] / (B * P2) + ALPHA * tot[C_BESTM] / (B * 2 * len(MARK))

    return np.array(kld_loss + ae_loss + best_mse + bias_loss, dtype=np.float32)
